# revision 4
# baseline (speedup 1.0000x reference)
"""Bidirectional column-chained GRU (vertical BiGRU over image columns) on 8 Trainium2 cores.

Topology: cores 0-3 run the forward GRU chain (batch quarters), cores 4-7 the
backward chain (rows pre-reversed on host). Each core runs the full C*S=16384
sequential GRU steps for its 8 batch rows in feature-major layout (128
partitions = hidden dim, free dim = batch).

Math restructuring (validated vs reference in numpy + CoreSim):
  state hp1 = h + 1; tanh(x) = 2*sigmoid(2x) - 1 (single ACT table);
  per half-column the rank-1 input contribution A_g,t = Wih_g*x_t + const_g
  is preloaded into PSUM with K=2 matmuls (x row + on-device ones row); the
  recurrent matmul Whh_g @ hp1 accumulates per step into PSUM slice t.
  Per step: r=sig(ps_r) [ACT], u=sig(-ps_z) [ACT], q=r*ps_n, w=q+a_n [DVE],
  v=sig(2w) [ACT], e1=u*hp1, f=hp1-e1, e2=2uv, hp1'=f+e2 [DVE]. The PE
  matmuls read [f, e2] directly (linearity) so the state add is off the
  recurrent critical path.

v2 over the original baseline:
  - Inputs packed into 4 DRAM tensors per core (~750KB vs ~1.5MB over 8):
    xcols (x columns, ones row built on device), wpack (whh^T | wfc^T | hp1_0),
    lcat (preload lhsT), wrows (b_fc row; broadcast on device by a K=1 ones
    matmul). Halves the axon transfer per call.
  - a_n PSUM->SBUF copy moved from ACT to DVE: the ACT table stays pinned to
    Sigmoid through the whole column loop (no LoadActFuncSet switches).
  - Static (non-rotating) xa/PSUM preload tiles, software-pipelined: the next
    column's x DMA and half-0 preload matmuls + a_n copies are emitted one per
    step window inside the current column's steps, so column boundaries no
    longer stall PE/DVE (boundary cost ~8us -> ~2.7us per column in sim).
  - Per-step scratch tiles (r/u/q/w/v/e1) and the loop-carried [fp|e2]/hp1
    state are parity double-buffered (t%2): WAR hazards move two steps out,
    which drops several cross-engine wait conditions from the serial chain
    (~77ns/step in sim). x input ships as bf16 (upconverted on device), the
    softmax output returns as bf16; jax persistent compile cache avoids the
    ~0.5s/call client-side recompile.
"""

import numpy as np
import jax
import ml_dtypes

# Persistent caches: the XLA-level executable (and the NEFF inside it) are
# content-addressed, so repeat calls — and fresh processes on the same box —
# skip the ~0.5s client-side BIR verify/DVE-table/compile path per call.
jax.config.update("jax_compilation_cache_dir", "/tmp/jax_cache")
jax.config.update("jax_persistent_cache_min_entry_size_bytes", -1)
jax.config.update("jax_persistent_cache_min_compile_time_secs", 0)

import concourse.bass as bass
import concourse.bacc as bacc
import concourse.mybir as mybir
import concourse.tile as tile
from concourse.bass_utils import run_bass_kernel_spmd

B, S, C, H, O = 32, 128, 128, 128, 64
NCORES = 8
BL = B // 4          # batch rows per core (4 groups x 2 directions)
SB = S * BL          # rhs columns per image column
HS = SB // 2         # half-column psum width (one bank)
NSTEP = S // 2       # steps per half
f32 = mybir.dt.float32
bf16 = mybir.dt.bfloat16
FP = mybir.EngineType

# wpack column layout
WC_WHH = 0           # 0:384   whhrT | whhzT | whhnT
WC_WFC = 384         # 384:448 wfcT
WC_HP1 = 448         # 448:456 initial hp1 (h_prev + 1, transposed)
WP = 456
# wrows [1, 1536]: lcat_w | lcat_c | b_fc tiled 8x (all on partition 0)


def _emit(nc: bacc.Bacc, n_cols: int = C, loop_cols: int | None = None,
          skip_collective: bool = False, zero_hall: bool = False,
          sph: int = NSTEP, zs: bool = True):
    AF = mybir.ActivationFunctionType
    OPM = mybir.AluOpType.mult

    xcols_d = nc.dram_tensor("xcols", [n_cols + 1, SB], bf16, kind="ExternalInput").ap()
    wpack_d = nc.dram_tensor("wpack", [H, WP], f32, kind="ExternalInput").ap()
    wrows_d = nc.dram_tensor("wrows", [1, 3 * 512], f32, kind="ExternalInput").ap()
    lcat_d = nc.dram_tensor("lcat", [2, 4 * H], f32, kind="ExternalInput").ap()
    out_d = nc.dram_tensor("out", [C * BL, O], bf16, kind="ExternalOutput").ap()

    with tile.TileContext(nc) as tc:
        with tc.tile_pool(name="const", bufs=1) as cp:
            wpack = cp.tile([H, WP], f32)
            wrows = cp.tile([1, 3 * 512], f32)
            lcat = cp.tile([2, 4 * H], f32)
            hp1 = cp.tile([H, BL], f32)
            hall = cp.tile([H, C * BL], f32)
            r = cp.tile([H, 2 * BL], f32)
            u = cp.tile([H, 2 * BL], f32)
            q = cp.tile([H, 2 * BL], f32)
            w = cp.tile([H, 2 * BL], f32)
            v = cp.tile([H, 2 * BL], f32)
            e1 = cp.tile([H, 2 * BL], f32)
            if zs:
                fe2 = cp.tile([H, 4 * BL], f32)
                fp_, e2 = fe2[:, 0:BL], fe2[:, BL : 2 * BL]
            else:
                fp_ = cp.tile([H, BL], f32)
                e2 = cp.tile([H, BL], f32)
            hp2 = cp.tile([H, 2 * BL], f32)

            whhrT = wpack[:, 0:H]
            whhzT = wpack[:, H : 2 * H]
            whhnT = wpack[:, 2 * H : 3 * H]
            wfcT = wpack[:, WC_WFC : WC_WFC + O]
            bfc8 = wrows[0:1, 1024 : 1024 + 8 * O]

            if zero_hall:
                nc.gpsimd.memset(hall[:], 0.0)
            nc.sync.dma_start(wpack[:], wpack_d)
            nc.sync.dma_start(wrows[:], wrows_d)
            nc.sync.dma_start(lcat[:], lcat_d)
            nc.vector.tensor_scalar_add(hp1[:], wpack[:, WC_HP1 : WC_HP1 + BL], 0.0)
            nc.vector.tensor_scalar_add(fp_[:], wpack[:, WC_HP1 : WC_HP1 + BL], 0.0)
            nc.vector.memzero(e2[:])
            if zs:
                nc.vector.tensor_scalar_add(
                    hp2[:, 0:BL], wpack[:, WC_HP1 : WC_HP1 + BL], 0.0)

            xa = cp.tile([2, SB], f32)
            xbf = cp.tile([1, SB], bf16)
            a_n0 = cp.tile([H, HS], f32)
            a_n1 = cp.tile([H, HS], f32)
            nc.gpsimd.memset(xa[:], 1.0)
            nc.sync.dma_start(xbf[:], xcols_d[0:1, :])
            nc.vector.tensor_scalar_add(xa[0:1, :], xbf[:], 0.0)

            with (
                tc.tile_pool(name="ps", bufs=1, space="PSUM") as psp,
            ):
                pst = {}
                for half, a_n in ((0, a_n0), (1, a_n1)):
                    pst[half] = tuple(
                        psp.tile([H, HS], f32, name=f"ps_{g}{half}")
                        for g in "rznt"
                    ) + (a_n,)

                def preload_mm(half, g):
                    ps = pst[half][g]
                    xh = xa[:, half * HS : (half + 1) * HS]
                    nc.tensor.matmul(
                        ps[:], lcat[:, g * H : (g + 1) * H], xh,
                        start=True, stop=True,
                    )

                def preload_copy(half, chunk, nchunk=2):
                    cw = HS // nchunk
                    csl = slice(chunk * cw, (chunk + 1) * cw)
                    nc.vector.tensor_scalar_add(
                        pst[half][4][:, csl], pst[half][3][:, csl], 0.0
                    )

                def preload_all(half):
                    for g in range(4):
                        preload_mm(half, g)
                    preload_copy(half, 0)
                    preload_copy(half, 1)

                # column 0's first half is preloaded before the loop
                preload_all(0)

                with tc.For_i(
                    0, n_cols if loop_cols is None else loop_cols, 1,
                    hint_engines=(FP.PE, FP.Activation, FP.DVE),
                ) as cv:

                    def steps(half, lo, hi, post=None):
                        ps_r, ps_z, ps_n, _, a_n = pst[half]
                        for t in range(lo, hi):
                            sl = slice(t * BL, (t + 1) * BL)
                            pb = slice((t % 2) * BL, (t % 2 + 1) * BL)
                            rr, uu, qq, ww, vv, ee1 = (
                                x[:, pb] for x in (r, u, q, w, v, e1))
                            pr, pw = t % 2, (t + 1) % 2
                            if zs:
                                fpw = fe2[:, pw * 2 * BL : pw * 2 * BL + BL]
                                e2w = fe2[:, pw * 2 * BL + BL : (pw + 1) * 2 * BL]
                                hpr = hp2[:, pr * BL : (pr + 1) * BL]
                                hpw = hp2[:, pw * BL : (pw + 1) * BL]
                                hp1v = fe2[
                                    :, pr * 2 * BL : (pr + 1) * 2 * BL
                                ].rearrange("p (a o) -> p a o", a=2)
                                outs = [
                                    bass.broadcast_tensor_aps(
                                        ps[:, sl].rearrange("p (a o) -> p a o", a=1),
                                        hp1v,
                                    )[0]
                                    for ps in (ps_r, ps_n, ps_z)
                                ]
                            else:
                                fpw, e2w, hpr, hpw = fp_[:], e2[:], hp1[:], hp1[:]
                                hp1v = hp1[:]
                                outs = [ps_r[:, sl], ps_n[:, sl], ps_z[:, sl]]
                            for o_, w_ in zip(outs, (whhrT, whhnT, whhzT)):
                                nc.tensor.matmul(
                                    o_, w_, hp1v, start=False, stop=True,
                                    skip_group_check=True,
                                )
                            nc.scalar.activation(rr, ps_r[:, sl], AF.Sigmoid)
                            nc.scalar.activation(uu, ps_z[:, sl], AF.Sigmoid, scale=-1.0)
                            nc.vector.tensor_mul(qq, rr, ps_n[:, sl])
                            nc.vector.tensor_add(ww, qq, a_n[:, sl])
                            nc.scalar.activation(vv, ww, AF.Sigmoid, scale=2.0)
                            nc.vector.tensor_mul(ee1, uu, hpr)
                            nc.vector.tensor_sub(fpw, hpr, ee1)
                            nc.vector.scalar_tensor_tensor(
                                e2w, uu, 2.0, vv, op0=OPM, op1=OPM
                            )
                            nc.vector.tensor_add(hpw, fpw, e2w)
                            if post and t in post:
                                for thunk in post[t]:
                                    thunk()

                    # interleave half-1 preloads into half-0's step windows,
                    # and next column's x DMA + half-0 preloads into half-1's.
                    p0 = {
                        8 + i: [lambda g=i: preload_mm(1, g)] for i in range(4)
                    }
                    p0[13] = [lambda: preload_copy(1, 0)]
                    p0[15] = [lambda: preload_copy(1, 1)]
                    p1 = {
                        16: [lambda: nc.sync.dma_start(
                            xbf[:], xcols_d[bass.ds(cv + 1, 1), :]
                        )],
                        18: [lambda: nc.vector.tensor_scalar_add(
                            xa[0:1, :], xbf[:], 0.0
                        )],
                    }
                    for i in range(4):
                        p1[24 + 2 * i] = [lambda g=i: preload_mm(0, g)]
                    p1[34] = [lambda: preload_copy(0, 0)]
                    p1[36] = [lambda: preload_copy(0, 1)]

                    steps(0, 0, sph, post=p0)
                    steps(1, 0, sph, post=p1)
                    nc.vector.tensor_scalar_add(
                        hall[:, bass.ts(cv, BL)],
                        hp2[:, 0:BL] if zs else hp1[:], -1.0
                    )

            # output head: partial logits -> allreduce(fwd,bwd) -> softmax(relu(.))
            with (
                tc.tile_pool(name="fc", bufs=1) as fcp,
                tc.tile_pool(name="psfc", bufs=1, space="PSUM") as psfc,
                tc.tile_pool(name="dramp", bufs=1, space="DRAM") as dp,
            ):
                lps = psfc.tile([128, 8 * O], f32)
                for k in range(8):
                    nc.tensor.matmul(
                        lps[:, k * O : (k + 1) * O],
                        hall[:, k * 128 : (k + 1) * 128],
                        wfcT,
                        start=True,
                        stop=True,
                    )
                lsb = fcp.tile([128, 8 * O], f32)
                nc.vector.tensor_scalar_add(lsb[:], lps[:], 0.0)
                lloc = dp.tile([C * BL, O], f32)
                lred = dp.tile([C * BL, O], f32)
                nc.sync.dma_start(
                    lloc.rearrange("(k p) o -> p k o", p=128),
                    lsb[:].rearrange("p (k o) -> p k o", k=8),
                )
                if skip_collective:
                    nc.sync.dma_start(lred[:], lloc[:])
                else:
                    nc.gpsimd.collective_compute(
                        "AllReduce",
                        mybir.AluOpType.add,
                        replica_groups=[[0, 4], [1, 5], [2, 6], [3, 7]],
                        ins=[lloc.opt()],
                        outs=[lred.opt()],
                    )
                # bias broadcast: ones128^T (128,1) @ bfc8 (1, 512)
                bias_ps = psfc.tile([128, 8 * O], f32)
                ones128 = fcp.tile([1, H], f32)
                nc.gpsimd.memset(ones128[:], 1.0)
                nc.tensor.matmul(bias_ps[:], ones128[:], bfc8, start=True, stop=True)
                lsum = fcp.tile([128, 8 * O], f32)
                nc.sync.dma_start(
                    lsum[:].rearrange("p (k o) -> p k o", k=8),
                    lred.rearrange("(k p) o -> p k o", p=128),
                )
                lbi = fcp.tile([128, 8 * O], f32)
                nc.vector.tensor_add(lbi[:], lsum[:], bias_ps[:])
                ex = fcp.tile([128, 8 * O], f32)
                nc.scalar.activation(ex[:], lbi[:], AF.Exp)
                # exp(relu(x)) == max(1, exp(x))
                nc.vector.tensor_scalar_max(ex[:], ex[:], 1.0)
                sums = fcp.tile([128, 8], f32)
                nc.vector.tensor_reduce(
                    sums[:],
                    ex[:].rearrange("p (k o) -> p k o", k=8),
                    axis=mybir.AxisListType.X,
                    op=mybir.AluOpType.add,
                )
                rs = fcp.tile([128, 8], f32)
                nc.vector.reciprocal(rs[:], sums[:])
                osb = fcp.tile([128, 8 * O], bf16)
                for k in range(8):
                    nc.vector.tensor_scalar_mul(
                        osb[:, k * O : (k + 1) * O],
                        ex[:, k * O : (k + 1) * O],
                        rs[:, k : k + 1],
                    )
                nc.sync.dma_start(
                    out_d.rearrange("(k p) o -> p k o", p=128),
                    osb[:].rearrange("p (k o) -> p k o", k=8),
                )


_CACHE = {}


def _build():
    if "nc" not in _CACHE:
        nc = bacc.Bacc("TRN2", target_bir_lowering=False, debug=False, num_devices=NCORES)
        _emit(nc)
        nc.compile()
        _CACHE["nc"] = nc
    return _CACHE["nc"]


def _dir_prep(inputs, d):
    """Per-direction host prep shared by the 4 batch-group cores."""
    sfx = "f" if d == 0 else "b"
    Wih = inputs[f"Wih_{sfx}"][:, 0]
    Whh = inputs[f"Whh_{sfx}"]
    bih = inputs[f"bih_{sfx}"]
    bhh = inputs[f"bhh_{sfx}"]
    Wr, Wz, Wn = Whh[:H], Whh[H : 2 * H], Whh[2 * H :]
    lcat = np.zeros((2, 4 * H), np.float32)
    lcat[0, 0:H] = Wih[:H]
    lcat[1, 0:H] = bih[:H] + bhh[:H] - Wr.sum(1)
    lcat[0, H : 2 * H] = Wih[H : 2 * H]
    lcat[1, H : 2 * H] = bih[H : 2 * H] + bhh[H : 2 * H] - Wz.sum(1)
    lcat[1, 2 * H : 3 * H] = bhh[2 * H :] - Wn.sum(1)
    lcat[0, 3 * H : 4 * H] = Wih[2 * H :]
    lcat[1, 3 * H : 4 * H] = bih[2 * H :]
    wfc_half = inputs["W_fc"][:, :H] if d == 0 else inputs["W_fc"][:, H:]
    wpack = np.zeros((H, WP), np.float32)
    wpack[:, 0:H] = Wr.T
    wpack[:, H : 2 * H] = Wz.T
    wpack[:, 2 * H : 3 * H] = Wn.T
    wpack[:, WC_WFC : WC_WFC + O] = wfc_half.T
    wrows = np.zeros((1, 3 * 512), np.float32)
    wrows[0, 1024 : 1024 + 8 * O] = np.tile(inputs["b_fc"], 8)
    xT = np.transpose(inputs["x"], (2, 1, 0))  # (C, S, B) view
    if d == 1:
        xT = xT[:, ::-1, :]
    return {"wpack": wpack, "wrows": wrows, "lcat": lcat, "xT": xT}


def _core_inputs(inputs, d, g, prep=None):
    """Host-side prep for core (direction d, batch group g)."""
    if prep is None:
        prep = _dir_prep(inputs, d)
    bsl = slice(g * BL, (g + 1) * BL)
    xcols_pad = np.zeros((C + 1, SB), ml_dtypes.bfloat16)
    xcols_pad[:C] = np.asarray(
        prep["xT"][:, :, bsl], dtype=ml_dtypes.bfloat16
    ).reshape(C, SB)
    wpack = prep["wpack"].copy()
    wpack[:, WC_HP1 : WC_HP1 + BL] = (inputs["h_prev"][d, bsl] + 1.0).T
    return {"xcols": xcols_pad, "wpack": wpack, "wrows": prep["wrows"],
            "lcat": prep["lcat"]}


def kernel(**inputs) -> np.ndarray:
    inputs = {k: np.asarray(v, dtype=np.float32) for k, v in inputs.items()}
    nc = _build()
    preps = {d: _dir_prep(inputs, d) for d in (0, 1)}
    in_maps = []
    for core in range(NCORES):
        d, g = (0, core) if core < 4 else (1, core - 4)
        in_maps.append(_core_inputs(inputs, d, g, preps[d]))
    res = run_bass_kernel_spmd(nc, in_maps, core_ids=list(range(NCORES)))
    out = np.empty((B, C, O), np.float32)
    for g in range(4):
        o = res.results[g]["out"].astype(np.float32).reshape(C, BL, O)
        out[g * BL : (g + 1) * BL] = np.transpose(o, (1, 0, 2))
    return out



# revision 5
# speedup vs baseline: 1.0346x; 1.0346x over previous
"""Bidirectional column-chained GRU (vertical BiGRU over image columns) on 8 Trainium2 cores.

Topology: cores 0-3 run the forward GRU chain (batch quarters), cores 4-7 the
backward chain (rows pre-reversed on host). Each core runs the full C*S=16384
sequential GRU steps for its 8 batch rows in feature-major layout (128
partitions = hidden dim, free dim = batch).

Math restructuring (validated vs reference in numpy + CoreSim):
  state hp1 = h + 1; tanh(x) = 2*sigmoid(2x) - 1 (single ACT table);
  per half-column the rank-1 input contribution A_g,t = Wih_g*x_t + const_g
  is preloaded into PSUM with K=2 matmuls (x row + on-device ones row); the
  recurrent matmul Whh_g @ hp1 accumulates per step into PSUM slice t.
  Per step: r=sig(ps_r) [ACT], u=sig(-ps_z) [ACT], q=r*ps_n, w=q+a_n [DVE],
  v=sig(2w) [ACT], e1=u*hp1, f=hp1-e1, e2=2uv, hp1'=f+e2 [DVE]. The PE
  matmuls read [f, e2] directly (linearity) so the state add is off the
  recurrent critical path.

v2 over the original baseline:
  - Inputs packed into 4 DRAM tensors per core (~750KB vs ~1.5MB over 8):
    xcols (x columns, ones row built on device), wpack (whh^T | wfc^T | hp1_0),
    lcat (preload lhsT), wrows (b_fc row; broadcast on device by a K=1 ones
    matmul). Halves the axon transfer per call.
  - a_n PSUM->SBUF copy moved from ACT to DVE: the ACT table stays pinned to
    Sigmoid through the whole column loop (no LoadActFuncSet switches).
  - Static (non-rotating) xa/PSUM preload tiles, software-pipelined: the next
    column's x DMA and half-0 preload matmuls + a_n copies are emitted one per
    step window inside the current column's steps, so column boundaries no
    longer stall PE/DVE (boundary cost ~8us -> ~2.7us per column in sim).
  - Per-step scratch tiles (r/u/q/w/v/e1) and the loop-carried [fp|e2]/hp1
    state are parity double-buffered (t%2): WAR hazards move two steps out,
    which drops several cross-engine wait conditions from the serial chain
    (~77ns/step in sim). x input ships as bf16 (upconverted on device), the
    softmax output returns as bf16; jax persistent compile cache avoids the
    ~0.5s/call client-side recompile.
"""

import numpy as np
import jax
import ml_dtypes

# Persistent caches: the XLA-level executable (and the NEFF inside it) are
# content-addressed, so repeat calls — and fresh processes on the same box —
# skip the ~0.5s client-side BIR verify/DVE-table/compile path per call.
jax.config.update("jax_compilation_cache_dir", "/tmp/jax_cache")
jax.config.update("jax_persistent_cache_min_entry_size_bytes", -1)
jax.config.update("jax_persistent_cache_min_compile_time_secs", 0)

import concourse.bass as bass
import concourse.bacc as bacc
import concourse.mybir as mybir
import concourse.tile as tile
from concourse.bass_utils import run_bass_kernel_spmd

B, S, C, H, O = 32, 128, 128, 128, 64
NCORES = 8
BL = B // 4          # batch rows per core (4 groups x 2 directions)
SB = S * BL          # rhs columns per image column
HS = SB // 2         # half-column psum width (one bank)
NSTEP = S // 2       # steps per half
f32 = mybir.dt.float32
bf16 = mybir.dt.bfloat16
FP = mybir.EngineType

# wpack column layout
WC_WHH = 0           # 0:384   whhrT | whhzT | whhnT
WC_WFC = 384         # 384:448 wfcT
WC_HP1 = 448         # 448:456 initial hp1 (h_prev + 1, transposed)
WP = 456
# wrows [1, 1536]: lcat_w | lcat_c | b_fc tiled 8x (all on partition 0)


def _emit(nc: bacc.Bacc, n_cols: int = C, loop_cols: int | None = None,
          skip_collective: bool = False, zero_hall: bool = False,
          sph: int = NSTEP, zs: bool = True):
    AF = mybir.ActivationFunctionType
    OPM = mybir.AluOpType.mult

    xcols_d = nc.dram_tensor("xcols", [n_cols + 1, SB], bf16, kind="ExternalInput").ap()
    wpack_d = nc.dram_tensor("wpack", [H, WP], f32, kind="ExternalInput").ap()
    wrows_d = nc.dram_tensor("wrows", [1, 3 * 512], f32, kind="ExternalInput").ap()
    lcat_d = nc.dram_tensor("lcat", [2, 4 * H], f32, kind="ExternalInput").ap()
    whhb_d = nc.dram_tensor("whhb", [H, 3 * H], bf16, kind="ExternalInput").ap()
    out_d = nc.dram_tensor("out", [C * BL, O], bf16, kind="ExternalOutput").ap()

    with tile.TileContext(nc) as tc:
        with tc.tile_pool(name="const", bufs=1) as cp:
            wpack = cp.tile([H, WP], f32)
            whhb = cp.tile([H, 3 * H], bf16)
            wrows = cp.tile([1, 3 * 512], f32)
            lcat = cp.tile([2, 4 * H], f32)
            hp1 = cp.tile([H, BL], f32)
            hall = cp.tile([H, C * BL], f32)
            r = cp.tile([H, 2 * BL], f32)
            u = cp.tile([H, 2 * BL], f32)
            q = cp.tile([H, 2 * BL], f32)
            w = cp.tile([H, 2 * BL], f32)
            v = cp.tile([H, 2 * BL], f32)
            e1 = cp.tile([H, 2 * BL], f32)
            if zs:
                fe2 = cp.tile([H, 4 * BL], bf16)
                fp_, e2 = fe2[:, 0:BL], fe2[:, BL : 2 * BL]
            else:
                fp_ = cp.tile([H, BL], f32)
                e2 = cp.tile([H, BL], f32)
            hp2 = cp.tile([H, 2 * BL], f32)

            whhrT = wpack[:, 0:H]
            whhzT = wpack[:, H : 2 * H]
            whhnT = wpack[:, 2 * H : 3 * H]
            wfcT = wpack[:, WC_WFC : WC_WFC + O]
            bfc8 = wrows[0:1, 1024 : 1024 + 8 * O]

            if zero_hall:
                nc.gpsimd.memset(hall[:], 0.0)
            nc.sync.dma_start(wpack[:], wpack_d)
            nc.sync.dma_start(whhb[:], whhb_d)
            nc.sync.dma_start(wrows[:], wrows_d)
            nc.sync.dma_start(lcat[:], lcat_d)
            nc.vector.tensor_scalar_add(hp1[:], wpack[:, WC_HP1 : WC_HP1 + BL], 0.0)
            nc.vector.tensor_scalar_add(fp_[:], wpack[:, WC_HP1 : WC_HP1 + BL], 0.0)
            nc.vector.memzero(e2[:])
            if zs:
                nc.vector.tensor_scalar_add(
                    hp2[:, 0:BL], wpack[:, WC_HP1 : WC_HP1 + BL], 0.0)

            xa = cp.tile([2, SB], f32)
            xbf = cp.tile([1, SB], bf16)
            a_n0 = cp.tile([H, HS], f32)
            a_n1 = cp.tile([H, HS], f32)
            nc.gpsimd.memset(xa[:], 1.0)
            nc.sync.dma_start(xbf[:], xcols_d[0:1, :])
            nc.vector.tensor_scalar_add(xa[0:1, :], xbf[:], 0.0)

            with (
                tc.tile_pool(name="ps", bufs=1, space="PSUM") as psp,
            ):
                pst = {}
                for half, a_n in ((0, a_n0), (1, a_n1)):
                    pst[half] = tuple(
                        psp.tile([H, HS], f32, name=f"ps_{g}{half}")
                        for g in "rznt"
                    ) + (a_n,)

                def preload_mm(half, g):
                    ps = pst[half][g]
                    xh = xa[:, half * HS : (half + 1) * HS]
                    nc.tensor.matmul(
                        ps[:], lcat[:, g * H : (g + 1) * H], xh,
                        start=True, stop=True,
                    )

                def preload_copy(half, chunk, nchunk=2):
                    cw = HS // nchunk
                    csl = slice(chunk * cw, (chunk + 1) * cw)
                    nc.vector.tensor_scalar_add(
                        pst[half][4][:, csl], pst[half][3][:, csl], 0.0
                    )

                def preload_all(half):
                    for g in range(4):
                        preload_mm(half, g)
                    preload_copy(half, 0)
                    preload_copy(half, 1)

                # column 0's first half is preloaded before the loop
                preload_all(0)

                with tc.For_i(
                    0, n_cols if loop_cols is None else loop_cols, 1,
                    hint_engines=(FP.PE, FP.Activation, FP.DVE),
                ) as cv:

                    def steps(half, lo, hi, post=None):
                        ps_r, ps_z, ps_n, _, a_n = pst[half]
                        for t in range(lo, hi):
                            sl = slice(t * BL, (t + 1) * BL)
                            pb = slice((t % 2) * BL, (t % 2 + 1) * BL)
                            rr, uu, qq, ww, vv, ee1 = (
                                x[:, pb] for x in (r, u, q, w, v, e1))
                            pr, pw = t % 2, (t + 1) % 2
                            if zs:
                                fpw = fe2[:, pw * 2 * BL : pw * 2 * BL + BL]
                                e2w = fe2[:, pw * 2 * BL + BL : (pw + 1) * 2 * BL]
                                hpr = hp2[:, pr * BL : (pr + 1) * BL]
                                hpw = hp2[:, pw * BL : (pw + 1) * BL]
                                hp1v = fe2[
                                    :, pr * 2 * BL : (pr + 1) * 2 * BL
                                ].rearrange("p (a o) -> p a o", a=2)
                                outs = [
                                    bass.broadcast_tensor_aps(
                                        ps[:, sl].rearrange("p (a o) -> p a o", a=1),
                                        hp1v,
                                    )[0]
                                    for ps in (ps_r, ps_n, ps_z)
                                ]
                            else:
                                fpw, e2w, hpr, hpw = fp_[:], e2[:], hp1[:], hp1[:]
                                hp1v = hp1[:]
                                outs = [ps_r[:, sl], ps_n[:, sl], ps_z[:, sl]]
                            if zs:
                                wsel = (whhb[:, 0:H], whhb[:, 2 * H : 3 * H],
                                        whhb[:, H : 2 * H])
                            else:
                                wsel = (whhrT, whhnT, whhzT)
                            for o_, w_ in zip(outs, wsel):
                                nc.tensor.matmul(
                                    o_, w_, hp1v, start=False, stop=True,
                                    skip_group_check=True,
                                )
                            nc.scalar.activation(rr, ps_r[:, sl], AF.Sigmoid)
                            nc.scalar.activation(uu, ps_z[:, sl], AF.Sigmoid, scale=-1.0)
                            nc.vector.tensor_mul(qq, rr, ps_n[:, sl])
                            nc.vector.tensor_add(ww, qq, a_n[:, sl])
                            nc.scalar.activation(vv, ww, AF.Sigmoid, scale=2.0)
                            nc.vector.tensor_mul(ee1, uu, hpr)
                            nc.vector.tensor_sub(fpw, hpr, ee1)
                            nc.vector.scalar_tensor_tensor(
                                e2w, uu, 2.0, vv, op0=OPM, op1=OPM
                            )
                            nc.vector.tensor_add(hpw, fpw, e2w)
                            if post and t in post:
                                for thunk in post[t]:
                                    thunk()

                    # interleave half-1 preloads into half-0's step windows,
                    # and next column's x DMA + half-0 preloads into half-1's.
                    p0 = {
                        8 + i: [lambda g=i: preload_mm(1, g)] for i in range(4)
                    }
                    p0[13] = [lambda: preload_copy(1, 0)]
                    p0[15] = [lambda: preload_copy(1, 1)]
                    p1 = {
                        16: [lambda: nc.sync.dma_start(
                            xbf[:], xcols_d[bass.ds(cv + 1, 1), :]
                        )],
                        18: [lambda: nc.vector.tensor_scalar_add(
                            xa[0:1, :], xbf[:], 0.0
                        )],
                    }
                    for i in range(4):
                        p1[24 + 2 * i] = [lambda g=i: preload_mm(0, g)]
                    p1[34] = [lambda: preload_copy(0, 0)]
                    p1[36] = [lambda: preload_copy(0, 1)]

                    steps(0, 0, sph, post=p0)
                    steps(1, 0, sph, post=p1)
                    nc.vector.tensor_scalar_add(
                        hall[:, bass.ts(cv, BL)],
                        hp2[:, 0:BL] if zs else hp1[:], -1.0
                    )

            # output head: partial logits -> allreduce(fwd,bwd) -> softmax(relu(.))
            with (
                tc.tile_pool(name="fc", bufs=1) as fcp,
                tc.tile_pool(name="psfc", bufs=1, space="PSUM") as psfc,
                tc.tile_pool(name="dramp", bufs=1, space="DRAM") as dp,
            ):
                lps = psfc.tile([128, 8 * O], f32)
                for k in range(8):
                    nc.tensor.matmul(
                        lps[:, k * O : (k + 1) * O],
                        hall[:, k * 128 : (k + 1) * 128],
                        wfcT,
                        start=True,
                        stop=True,
                    )
                lsb = fcp.tile([128, 8 * O], f32)
                nc.vector.tensor_scalar_add(lsb[:], lps[:], 0.0)
                lloc = dp.tile([C * BL, O], f32)
                lred = dp.tile([C * BL, O], f32)
                nc.sync.dma_start(
                    lloc.rearrange("(k p) o -> p k o", p=128),
                    lsb[:].rearrange("p (k o) -> p k o", k=8),
                )
                if skip_collective:
                    nc.sync.dma_start(lred[:], lloc[:])
                else:
                    nc.gpsimd.collective_compute(
                        "AllReduce",
                        mybir.AluOpType.add,
                        replica_groups=[[0, 4], [1, 5], [2, 6], [3, 7]],
                        ins=[lloc.opt()],
                        outs=[lred.opt()],
                    )
                # bias broadcast: ones128^T (128,1) @ bfc8 (1, 512)
                bias_ps = psfc.tile([128, 8 * O], f32)
                ones128 = fcp.tile([1, H], f32)
                nc.gpsimd.memset(ones128[:], 1.0)
                nc.tensor.matmul(bias_ps[:], ones128[:], bfc8, start=True, stop=True)
                lsum = fcp.tile([128, 8 * O], f32)
                nc.sync.dma_start(
                    lsum[:].rearrange("p (k o) -> p k o", k=8),
                    lred.rearrange("(k p) o -> p k o", p=128),
                )
                lbi = fcp.tile([128, 8 * O], f32)
                nc.vector.tensor_add(lbi[:], lsum[:], bias_ps[:])
                ex = fcp.tile([128, 8 * O], f32)
                nc.scalar.activation(ex[:], lbi[:], AF.Exp)
                # exp(relu(x)) == max(1, exp(x))
                nc.vector.tensor_scalar_max(ex[:], ex[:], 1.0)
                sums = fcp.tile([128, 8], f32)
                nc.vector.tensor_reduce(
                    sums[:],
                    ex[:].rearrange("p (k o) -> p k o", k=8),
                    axis=mybir.AxisListType.X,
                    op=mybir.AluOpType.add,
                )
                rs = fcp.tile([128, 8], f32)
                nc.vector.reciprocal(rs[:], sums[:])
                osb = fcp.tile([128, 8 * O], bf16)
                for k in range(8):
                    nc.vector.tensor_scalar_mul(
                        osb[:, k * O : (k + 1) * O],
                        ex[:, k * O : (k + 1) * O],
                        rs[:, k : k + 1],
                    )
                nc.sync.dma_start(
                    out_d.rearrange("(k p) o -> p k o", p=128),
                    osb[:].rearrange("p (k o) -> p k o", k=8),
                )


_CACHE = {}


def _build():
    if "nc" not in _CACHE:
        nc = bacc.Bacc("TRN2", target_bir_lowering=False, debug=False, num_devices=NCORES)
        _emit(nc)
        nc.compile()
        _CACHE["nc"] = nc
    return _CACHE["nc"]


def _dir_prep(inputs, d):
    """Per-direction host prep shared by the 4 batch-group cores."""
    sfx = "f" if d == 0 else "b"
    Wih = inputs[f"Wih_{sfx}"][:, 0]
    Whh = inputs[f"Whh_{sfx}"]
    bih = inputs[f"bih_{sfx}"]
    bhh = inputs[f"bhh_{sfx}"]
    Wr, Wz, Wn = Whh[:H], Whh[H : 2 * H], Whh[2 * H :]
    lcat = np.zeros((2, 4 * H), np.float32)
    lcat[0, 0:H] = Wih[:H]
    lcat[1, 0:H] = bih[:H] + bhh[:H] - Wr.sum(1)
    lcat[0, H : 2 * H] = Wih[H : 2 * H]
    lcat[1, H : 2 * H] = bih[H : 2 * H] + bhh[H : 2 * H] - Wz.sum(1)
    lcat[1, 2 * H : 3 * H] = bhh[2 * H :] - Wn.sum(1)
    lcat[0, 3 * H : 4 * H] = Wih[2 * H :]
    lcat[1, 3 * H : 4 * H] = bih[2 * H :]
    wfc_half = inputs["W_fc"][:, :H] if d == 0 else inputs["W_fc"][:, H:]
    wpack = np.zeros((H, WP), np.float32)
    wpack[:, 0:H] = Wr.T
    wpack[:, H : 2 * H] = Wz.T
    wpack[:, 2 * H : 3 * H] = Wn.T
    wpack[:, WC_WFC : WC_WFC + O] = wfc_half.T
    whhb = np.concatenate([Wr.T, Wz.T, Wn.T], axis=1).astype(ml_dtypes.bfloat16)
    wrows = np.zeros((1, 3 * 512), np.float32)
    wrows[0, 1024 : 1024 + 8 * O] = np.tile(inputs["b_fc"], 8)
    xT = np.transpose(inputs["x"], (2, 1, 0))  # (C, S, B) view
    if d == 1:
        xT = xT[:, ::-1, :]
    return {"wpack": wpack, "wrows": wrows, "lcat": lcat, "xT": xT,
            "whhb": whhb}


def _core_inputs(inputs, d, g, prep=None):
    """Host-side prep for core (direction d, batch group g)."""
    if prep is None:
        prep = _dir_prep(inputs, d)
    bsl = slice(g * BL, (g + 1) * BL)
    xcols_pad = np.zeros((C + 1, SB), ml_dtypes.bfloat16)
    xcols_pad[:C] = np.asarray(
        prep["xT"][:, :, bsl], dtype=ml_dtypes.bfloat16
    ).reshape(C, SB)
    wpack = prep["wpack"].copy()
    wpack[:, WC_HP1 : WC_HP1 + BL] = (inputs["h_prev"][d, bsl] + 1.0).T
    return {"xcols": xcols_pad, "wpack": wpack, "wrows": prep["wrows"],
            "lcat": prep["lcat"], "whhb": prep["whhb"]}


def kernel(**inputs) -> np.ndarray:
    inputs = {k: np.asarray(v, dtype=np.float32) for k, v in inputs.items()}
    nc = _build()
    preps = {d: _dir_prep(inputs, d) for d in (0, 1)}
    in_maps = []
    for core in range(NCORES):
        d, g = (0, core) if core < 4 else (1, core - 4)
        in_maps.append(_core_inputs(inputs, d, g, preps[d]))
    res = run_bass_kernel_spmd(nc, in_maps, core_ids=list(range(NCORES)))
    out = np.empty((B, C, O), np.float32)
    for g in range(4):
        o = res.results[g]["out"].astype(np.float32).reshape(C, BL, O)
        out[g * BL : (g + 1) * BL] = np.transpose(o, (1, 0, 2))
    return out



# revision 6
# speedup vs baseline: 1.1910x; 1.1512x over previous
"""Bidirectional column-chained GRU (vertical BiGRU over image columns) on 8 Trainium2 cores.

Topology: cores 0-3 run the forward GRU chain (batch quarters), cores 4-7 the
backward chain (rows pre-reversed on host). Each core runs the full C*S=16384
sequential GRU steps for its 8 batch rows in feature-major layout (128
partitions = hidden dim, free dim = batch).

Math restructuring (validated vs reference in numpy + CoreSim):
  state hp1 = h + 1; tanh(x) = 2*sigmoid(2x) - 1 (single ACT table);
  per half-column the rank-1 input contribution A_g,t = Wih_g*x_t + const_g
  is preloaded into PSUM with K=2 matmuls (x row + on-device ones row); the
  recurrent matmul Whh_g @ hp1 accumulates per step into PSUM slice t.
  Per step: r=sig(ps_r) [ACT], u=sig(-ps_z) [ACT], q=r*ps_n, w=q+a_n [DVE],
  v=sig(2w) [ACT], e1=u*hp1, f=hp1-e1, e2=2uv, hp1'=f+e2 [DVE]. The PE
  matmuls read [f, e2] directly (linearity) so the state add is off the
  recurrent critical path.

v2 over the original baseline:
  - Inputs packed into 4 DRAM tensors per core (~750KB vs ~1.5MB over 8):
    xcols (x columns, ones row built on device), wpack (whh^T | wfc^T | hp1_0),
    lcat (preload lhsT), wrows (b_fc row; broadcast on device by a K=1 ones
    matmul). Halves the axon transfer per call.
  - a_n PSUM->SBUF copy moved from ACT to DVE: the ACT table stays pinned to
    Sigmoid through the whole column loop (no LoadActFuncSet switches).
  - Static (non-rotating) xa/PSUM preload tiles, software-pipelined: the next
    column's x DMA and half-0 preload matmuls + a_n copies are emitted one per
    step window inside the current column's steps, so column boundaries no
    longer stall PE/DVE (boundary cost ~8us -> ~2.7us per column in sim).
  - Per-step scratch tiles (r/u/q/w/v/e1) and the loop-carried [fp|e2]/hp1
    state are parity double-buffered (t%2): WAR hazards move two steps out,
    which drops several cross-engine wait conditions from the serial chain
    (~77ns/step in sim). x input ships as bf16 (upconverted on device), the
    softmax output returns as bf16; jax persistent compile cache avoids the
    ~0.5s/call client-side recompile.
"""

import numpy as np
import jax
import ml_dtypes

# Persistent caches: the XLA-level executable (and the NEFF inside it) are
# content-addressed, so repeat calls — and fresh processes on the same box —
# skip the ~0.5s client-side BIR verify/DVE-table/compile path per call.
jax.config.update("jax_compilation_cache_dir", "/tmp/jax_cache")
jax.config.update("jax_persistent_cache_min_entry_size_bytes", -1)
jax.config.update("jax_persistent_cache_min_compile_time_secs", 0)

import concourse.bass as bass
import concourse.bacc as bacc
import concourse.mybir as mybir
import concourse.tile as tile
from concourse.bass_utils import run_bass_kernel_spmd

B, S, C, H, O = 32, 128, 128, 128, 64
NCORES = 8
BL = B // 4          # batch rows per core (4 groups x 2 directions)
SB = S * BL          # rhs columns per image column
HS = SB // 2         # half-column psum width (one bank)
NSTEP = S // 2       # steps per half
f32 = mybir.dt.float32
bf16 = mybir.dt.bfloat16
FP = mybir.EngineType

# wpack column layout (recurrent weights ship separately as bf16 in whhb)
WC_WFC = 0           # 0:64   wfcT
WC_HP1 = 64          # 64:72  initial hp1 (h_prev + 1, transposed)
WP = 72
# wrows [1, 1536]: lcat_w | lcat_c | b_fc tiled 8x (all on partition 0)


def _emit(nc: bacc.Bacc, n_cols: int = C, loop_cols: int | None = None,
          skip_collective: bool = False, zero_hall: bool = False,
          sph: int = NSTEP, zs: bool = True):
    AF = mybir.ActivationFunctionType
    OPM = mybir.AluOpType.mult

    xcols_d = nc.dram_tensor("xcols", [n_cols + 1, SB], bf16, kind="ExternalInput").ap()
    wpack_d = nc.dram_tensor("wpack", [H, WP], f32, kind="ExternalInput").ap()
    wrows_d = nc.dram_tensor("wrows", [1, 3 * 512], f32, kind="ExternalInput").ap()
    lcat_d = nc.dram_tensor("lcat", [2, 4 * H], f32, kind="ExternalInput").ap()
    whhb_d = nc.dram_tensor("whhb", [H, 3 * H], bf16, kind="ExternalInput").ap()
    out_d = nc.dram_tensor("out", [C * BL, O], bf16, kind="ExternalOutput").ap()

    with tile.TileContext(nc) as tc:
        with tc.tile_pool(name="const", bufs=1) as cp:
            wpack = cp.tile([H, WP], f32)
            whhb = cp.tile([H, 3 * H], bf16)
            wrows = cp.tile([1, 3 * 512], f32)
            lcat = cp.tile([2, 4 * H], f32)
            hp1 = cp.tile([H, BL], f32)
            hall = cp.tile([H, C * BL], f32)
            r = cp.tile([H, 2 * BL], f32)
            u = cp.tile([H, 2 * BL], f32)
            q = cp.tile([H, 2 * BL], f32)
            w = cp.tile([H, 2 * BL], f32)
            v = cp.tile([H, 2 * BL], f32)
            e1 = cp.tile([H, 2 * BL], f32)
            if zs:
                fe2 = cp.tile([H, 4 * BL], bf16)
                fp_, e2 = fe2[:, 0:BL], fe2[:, BL : 2 * BL]
            else:
                fp_ = cp.tile([H, BL], f32)
                e2 = cp.tile([H, BL], f32)
            hp2 = cp.tile([H, 2 * BL], f32)

            wfcT = wpack[:, WC_WFC : WC_WFC + O]
            if not zs:
                # sim/debug path runs the recurrence in f32: upconvert whhb
                whhf = cp.tile([H, 3 * H], f32)
                nc.vector.tensor_scalar_add(whhf[:], whhb[:], 0.0)
                whhrT = whhf[:, 0:H]
                whhzT = whhf[:, H : 2 * H]
                whhnT = whhf[:, 2 * H : 3 * H]
            bfc8 = wrows[0:1, 1024 : 1024 + 8 * O]

            if zero_hall:
                nc.gpsimd.memset(hall[:], 0.0)
            nc.sync.dma_start(wpack[:], wpack_d)
            nc.sync.dma_start(whhb[:], whhb_d)
            nc.sync.dma_start(wrows[:], wrows_d)
            nc.sync.dma_start(lcat[:], lcat_d)
            nc.vector.tensor_scalar_add(hp1[:], wpack[:, WC_HP1 : WC_HP1 + BL], 0.0)
            nc.vector.tensor_scalar_add(fp_[:], wpack[:, WC_HP1 : WC_HP1 + BL], 0.0)
            nc.vector.memzero(e2[:])
            if zs:
                nc.vector.tensor_scalar_add(
                    hp2[:, 0:BL], wpack[:, WC_HP1 : WC_HP1 + BL], 0.0)

            xa = cp.tile([2, SB], f32)
            xbf = cp.tile([1, SB], bf16)
            a_n0 = cp.tile([H, HS], f32)
            a_n1 = cp.tile([H, HS], f32)
            nc.gpsimd.memset(xa[:], 1.0)
            nc.sync.dma_start(xbf[:], xcols_d[0:1, :])
            nc.vector.tensor_scalar_add(xa[0:1, :], xbf[:], 0.0)

            with (
                tc.tile_pool(name="ps", bufs=1, space="PSUM") as psp,
            ):
                pst = {}
                for half, a_n in ((0, a_n0), (1, a_n1)):
                    pst[half] = tuple(
                        psp.tile([H, HS], f32, name=f"ps_{g}{half}")
                        for g in "rznt"
                    ) + (a_n,)

                def preload_mm(half, g):
                    ps = pst[half][g]
                    xh = xa[:, half * HS : (half + 1) * HS]
                    nc.tensor.matmul(
                        ps[:], lcat[:, g * H : (g + 1) * H], xh,
                        start=True, stop=True,
                    )

                def preload_copy(half, chunk, nchunk=2):
                    cw = HS // nchunk
                    csl = slice(chunk * cw, (chunk + 1) * cw)
                    nc.vector.tensor_scalar_add(
                        pst[half][4][:, csl], pst[half][3][:, csl], 0.0
                    )

                def preload_all(half):
                    for g in range(4):
                        preload_mm(half, g)
                    preload_copy(half, 0)
                    preload_copy(half, 1)

                # column 0's first half is preloaded before the loop
                preload_all(0)

                with tc.For_i(
                    0, n_cols if loop_cols is None else loop_cols, 1,
                    hint_engines=(FP.PE, FP.Activation, FP.DVE),
                ) as cv:

                    def steps(half, lo, hi, post=None):
                        ps_r, ps_z, ps_n, _, a_n = pst[half]
                        for t in range(lo, hi):
                            sl = slice(t * BL, (t + 1) * BL)
                            pb = slice((t % 2) * BL, (t % 2 + 1) * BL)
                            rr, uu, qq, ww, vv, ee1 = (
                                x[:, pb] for x in (r, u, q, w, v, e1))
                            pr, pw = t % 2, (t + 1) % 2
                            if zs:
                                fpw = fe2[:, pw * 2 * BL : pw * 2 * BL + BL]
                                e2w = fe2[:, pw * 2 * BL + BL : (pw + 1) * 2 * BL]
                                hpr = hp2[:, pr * BL : (pr + 1) * BL]
                                hpw = hp2[:, pw * BL : (pw + 1) * BL]
                                hp1v = fe2[
                                    :, pr * 2 * BL : (pr + 1) * 2 * BL
                                ].rearrange("p (a o) -> p a o", a=2)
                                outs = [
                                    bass.broadcast_tensor_aps(
                                        ps[:, sl].rearrange("p (a o) -> p a o", a=1),
                                        hp1v,
                                    )[0]
                                    for ps in (ps_r, ps_n, ps_z)
                                ]
                            else:
                                fpw, e2w, hpr, hpw = fp_[:], e2[:], hp1[:], hp1[:]
                                hp1v = hp1[:]
                                outs = [ps_r[:, sl], ps_n[:, sl], ps_z[:, sl]]
                            if zs:
                                wsel = (whhb[:, 0:H], whhb[:, 2 * H : 3 * H],
                                        whhb[:, H : 2 * H])
                            else:
                                wsel = (whhrT, whhnT, whhzT)
                            for o_, w_ in zip(outs, wsel):
                                nc.tensor.matmul(
                                    o_, w_, hp1v, start=False, stop=True,
                                    skip_group_check=True,
                                )
                            nc.scalar.activation(rr, ps_r[:, sl], AF.Sigmoid)
                            nc.scalar.activation(uu, ps_z[:, sl], AF.Sigmoid, scale=-1.0)
                            nc.vector.tensor_mul(qq, rr, ps_n[:, sl])
                            nc.vector.tensor_add(ww, qq, a_n[:, sl])
                            nc.scalar.activation(vv, ww, AF.Sigmoid, scale=2.0)
                            nc.vector.tensor_mul(ee1, uu, hpr)
                            nc.vector.tensor_sub(fpw, hpr, ee1)
                            nc.vector.scalar_tensor_tensor(
                                e2w, uu, 2.0, vv, op0=OPM, op1=OPM
                            )
                            nc.vector.tensor_add(hpw, fpw, e2w)
                            if post and t in post:
                                for thunk in post[t]:
                                    thunk()

                    # interleave half-1 preloads into half-0's step windows,
                    # and next column's x DMA + half-0 preloads into half-1's.
                    p0 = {
                        8 + i: [lambda g=i: preload_mm(1, g)] for i in range(4)
                    }
                    p0[13] = [lambda: preload_copy(1, 0)]
                    p0[15] = [lambda: preload_copy(1, 1)]
                    p1 = {
                        16: [lambda: nc.sync.dma_start(
                            xbf[:], xcols_d[bass.ds(cv + 1, 1), :]
                        )],
                        18: [lambda: nc.vector.tensor_scalar_add(
                            xa[0:1, :], xbf[:], 0.0
                        )],
                    }
                    for i in range(4):
                        p1[24 + 2 * i] = [lambda g=i: preload_mm(0, g)]
                    p1[34] = [lambda: preload_copy(0, 0)]
                    p1[36] = [lambda: preload_copy(0, 1)]

                    steps(0, 0, sph, post=p0)
                    steps(1, 0, sph, post=p1)
                    nc.vector.tensor_scalar_add(
                        hall[:, bass.ts(cv, BL)],
                        hp2[:, 0:BL] if zs else hp1[:], -1.0
                    )

            # output head: partial logits -> allreduce(fwd,bwd) -> softmax(relu(.))
            with (
                tc.tile_pool(name="fc", bufs=1) as fcp,
                tc.tile_pool(name="psfc", bufs=1, space="PSUM") as psfc,
                tc.tile_pool(name="dramp", bufs=1, space="DRAM") as dp,
            ):
                lps = psfc.tile([128, 8 * O], f32)
                for k in range(8):
                    nc.tensor.matmul(
                        lps[:, k * O : (k + 1) * O],
                        hall[:, k * 128 : (k + 1) * 128],
                        wfcT,
                        start=True,
                        stop=True,
                    )
                lsb = fcp.tile([128, 8 * O], f32)
                nc.vector.tensor_scalar_add(lsb[:], lps[:], 0.0)
                lloc = dp.tile([C * BL, O], f32)
                lred = dp.tile([C * BL, O], f32)
                nc.sync.dma_start(
                    lloc.rearrange("(k p) o -> p k o", p=128),
                    lsb[:].rearrange("p (k o) -> p k o", k=8),
                )
                if skip_collective:
                    nc.sync.dma_start(lred[:], lloc[:])
                else:
                    nc.gpsimd.collective_compute(
                        "AllReduce",
                        mybir.AluOpType.add,
                        replica_groups=[[0, 4], [1, 5], [2, 6], [3, 7]],
                        ins=[lloc.opt()],
                        outs=[lred.opt()],
                    )
                # bias broadcast: ones128^T (128,1) @ bfc8 (1, 512)
                bias_ps = psfc.tile([128, 8 * O], f32)
                ones128 = fcp.tile([1, H], f32)
                nc.gpsimd.memset(ones128[:], 1.0)
                nc.tensor.matmul(bias_ps[:], ones128[:], bfc8, start=True, stop=True)
                lsum = fcp.tile([128, 8 * O], f32)
                nc.sync.dma_start(
                    lsum[:].rearrange("p (k o) -> p k o", k=8),
                    lred.rearrange("(k p) o -> p k o", p=128),
                )
                lbi = fcp.tile([128, 8 * O], f32)
                nc.vector.tensor_add(lbi[:], lsum[:], bias_ps[:])
                ex = fcp.tile([128, 8 * O], f32)
                nc.scalar.activation(ex[:], lbi[:], AF.Exp)
                # exp(relu(x)) == max(1, exp(x))
                nc.vector.tensor_scalar_max(ex[:], ex[:], 1.0)
                sums = fcp.tile([128, 8], f32)
                nc.vector.tensor_reduce(
                    sums[:],
                    ex[:].rearrange("p (k o) -> p k o", k=8),
                    axis=mybir.AxisListType.X,
                    op=mybir.AluOpType.add,
                )
                rs = fcp.tile([128, 8], f32)
                nc.vector.reciprocal(rs[:], sums[:])
                osb = fcp.tile([128, 8 * O], bf16)
                for k in range(8):
                    nc.vector.tensor_scalar_mul(
                        osb[:, k * O : (k + 1) * O],
                        ex[:, k * O : (k + 1) * O],
                        rs[:, k : k + 1],
                    )
                nc.sync.dma_start(
                    out_d.rearrange("(k p) o -> p k o", p=128),
                    osb[:].rearrange("p (k o) -> p k o", k=8),
                )


_CACHE = {}


def _build():
    if "nc" not in _CACHE:
        nc = bacc.Bacc("TRN2", target_bir_lowering=False, debug=False, num_devices=NCORES)
        _emit(nc)
        nc.compile()
        _CACHE["nc"] = nc
    return _CACHE["nc"]


def _dir_prep(inputs, d):
    """Per-direction host prep shared by the 4 batch-group cores."""
    sfx = "f" if d == 0 else "b"
    Wih = inputs[f"Wih_{sfx}"][:, 0]
    Whh = inputs[f"Whh_{sfx}"]
    bih = inputs[f"bih_{sfx}"]
    bhh = inputs[f"bhh_{sfx}"]
    Wr, Wz, Wn = Whh[:H], Whh[H : 2 * H], Whh[2 * H :]
    lcat = np.zeros((2, 4 * H), np.float32)
    lcat[0, 0:H] = Wih[:H]
    lcat[1, 0:H] = bih[:H] + bhh[:H] - Wr.sum(1)
    lcat[0, H : 2 * H] = Wih[H : 2 * H]
    lcat[1, H : 2 * H] = bih[H : 2 * H] + bhh[H : 2 * H] - Wz.sum(1)
    lcat[1, 2 * H : 3 * H] = bhh[2 * H :] - Wn.sum(1)
    lcat[0, 3 * H : 4 * H] = Wih[2 * H :]
    lcat[1, 3 * H : 4 * H] = bih[2 * H :]
    wfc_half = inputs["W_fc"][:, :H] if d == 0 else inputs["W_fc"][:, H:]
    wpack = np.zeros((H, WP), np.float32)
    wpack[:, WC_WFC : WC_WFC + O] = wfc_half.T
    whhb = np.concatenate([Wr.T, Wz.T, Wn.T], axis=1).astype(ml_dtypes.bfloat16)
    wrows = np.zeros((1, 3 * 512), np.float32)
    wrows[0, 1024 : 1024 + 8 * O] = np.tile(inputs["b_fc"], 8)
    xT = np.transpose(inputs["x"], (2, 1, 0))  # (C, S, B) view
    if d == 1:
        xT = xT[:, ::-1, :]
    return {"wpack": wpack, "wrows": wrows, "lcat": lcat, "xT": xT,
            "whhb": whhb}


def _core_inputs(inputs, d, g, prep=None):
    """Host-side prep for core (direction d, batch group g)."""
    if prep is None:
        prep = _dir_prep(inputs, d)
    bsl = slice(g * BL, (g + 1) * BL)
    xcols_pad = np.zeros((C + 1, SB), ml_dtypes.bfloat16)
    xcols_pad[:C] = np.asarray(
        prep["xT"][:, :, bsl], dtype=ml_dtypes.bfloat16
    ).reshape(C, SB)
    wpack = prep["wpack"].copy()
    wpack[:, WC_HP1 : WC_HP1 + BL] = (inputs["h_prev"][d, bsl] + 1.0).T
    return {"xcols": xcols_pad, "wpack": wpack, "wrows": prep["wrows"],
            "lcat": prep["lcat"], "whhb": prep["whhb"]}


def kernel(**inputs) -> np.ndarray:
    inputs = {k: np.asarray(v, dtype=np.float32) for k, v in inputs.items()}
    nc = _build()
    preps = {d: _dir_prep(inputs, d) for d in (0, 1)}
    in_maps = []
    for core in range(NCORES):
        d, g = (0, core) if core < 4 else (1, core - 4)
        in_maps.append(_core_inputs(inputs, d, g, preps[d]))
    res = run_bass_kernel_spmd(nc, in_maps, core_ids=list(range(NCORES)))
    out = np.empty((B, C, O), np.float32)
    for g in range(4):
        o = res.results[g]["out"].astype(np.float32).reshape(C, BL, O)
        out[g * BL : (g + 1) * BL] = np.transpose(o, (1, 0, 2))
    return out



# revision 7
# speedup vs baseline: 1.2054x; 1.0121x over previous
"""Bidirectional column-chained GRU (vertical BiGRU over image columns) on 8 Trainium2 cores.

Topology: cores 0-3 run the forward GRU chain (batch quarters), cores 4-7 the
backward chain (rows pre-reversed on host). Each core runs the full C*S=16384
sequential GRU steps for its 8 batch rows in feature-major layout (128
partitions = hidden dim, free dim = batch).

Math restructuring (validated vs reference in numpy + CoreSim):
  state hp1 = h + 1; tanh(x) = 2*sigmoid(2x) - 1 (single ACT table);
  per half-column the rank-1 input contribution A_g,t = Wih_g*x_t + const_g
  is preloaded into PSUM with K=2 matmuls (x row + on-device ones row); the
  recurrent matmul Whh_g @ hp1 accumulates per step into PSUM slice t.
  Per step: r=sig(ps_r) [ACT], u=sig(-ps_z) [ACT], q=r*ps_n, w=q+a_n [DVE],
  v=sig(2w) [ACT], e1=u*hp1, f=hp1-e1, e2=2uv, hp1'=f+e2 [DVE]. The PE
  matmuls read [f, e2] directly (linearity) so the state add is off the
  recurrent critical path.

v2 over the original baseline:
  - Inputs packed into 4 DRAM tensors per core (~750KB vs ~1.5MB over 8):
    xcols (x columns, ones row built on device), wpack (whh^T | wfc^T | hp1_0),
    lcat (preload lhsT), wrows (b_fc row; broadcast on device by a K=1 ones
    matmul). Halves the axon transfer per call.
  - a_n PSUM->SBUF copy moved from ACT to DVE: the ACT table stays pinned to
    Sigmoid through the whole column loop (no LoadActFuncSet switches).
  - Static (non-rotating) xa/PSUM preload tiles, software-pipelined: the next
    column's x DMA and half-0 preload matmuls + a_n copies are emitted one per
    step window inside the current column's steps, so column boundaries no
    longer stall PE/DVE (boundary cost ~8us -> ~2.7us per column in sim).
  - Per-step scratch tiles (r/u/q/w/v/e1) and the loop-carried [fp|e2]/hp1
    state are parity double-buffered (t%2): WAR hazards move two steps out,
    which drops several cross-engine wait conditions from the serial chain
    (~77ns/step in sim). x input ships as bf16 (upconverted on device), the
    softmax output returns as bf16; jax persistent compile cache avoids the
    ~0.5s/call client-side recompile.
"""

import numpy as np
import jax
import ml_dtypes

# Persistent caches: the XLA-level executable (and the NEFF inside it) are
# content-addressed, so repeat calls — and fresh processes on the same box —
# skip the ~0.5s client-side BIR verify/DVE-table/compile path per call.
jax.config.update("jax_compilation_cache_dir", "/tmp/jax_cache")
jax.config.update("jax_persistent_cache_min_entry_size_bytes", -1)
jax.config.update("jax_persistent_cache_min_compile_time_secs", 0)

import concourse.bass as bass
import concourse.bacc as bacc
import concourse.mybir as mybir
import concourse.tile as tile
from concourse.bass_utils import run_bass_kernel_spmd

B, S, C, H, O = 32, 128, 128, 128, 64
NCORES = 8
BL = B // 4          # batch rows per core (4 groups x 2 directions)
SB = S * BL          # rhs columns per image column
HS = SB // 2         # half-column psum width (one bank)
NSTEP = S // 2       # steps per half
f32 = mybir.dt.float32
bf16 = mybir.dt.bfloat16
FP = mybir.EngineType

# wpack column layout (recurrent weights ship separately as bf16 in whhb)
WC_WFC = 0           # 0:64   wfcT
WC_HP1 = 64          # 64:72  initial hp1 (h_prev + 1, transposed)
WP = 72
# wrows [1, 1536]: lcat_w | lcat_c | b_fc tiled 8x (all on partition 0)


def _emit(nc: bacc.Bacc, n_cols: int = C, loop_cols: int | None = None,
          skip_collective: bool = False, zero_hall: bool = False,
          sph: int = NSTEP, zs: bool = True):
    AF = mybir.ActivationFunctionType
    OPM = mybir.AluOpType.mult

    xcols_d = nc.dram_tensor("xcols", [n_cols + 1, SB], bf16, kind="ExternalInput").ap()
    wpack_d = nc.dram_tensor("wpack", [H, WP], f32, kind="ExternalInput").ap()
    wrows_d = nc.dram_tensor("wrows", [1, 3 * 512], f32, kind="ExternalInput").ap()
    lcat_d = nc.dram_tensor("lcat", [2, 4 * H], f32, kind="ExternalInput").ap()
    whhb_d = nc.dram_tensor("whhb", [H, 3 * H], bf16, kind="ExternalInput").ap()
    out_d = nc.dram_tensor("out", [C * BL, O], bf16, kind="ExternalOutput").ap()

    with tile.TileContext(nc) as tc:
        with tc.tile_pool(name="const", bufs=1) as cp:
            wpack = cp.tile([H, WP], f32)
            whhb = cp.tile([H, 3 * H], bf16)
            wrows = cp.tile([1, 3 * 512], f32)
            lcat = cp.tile([2, 4 * H], f32)
            hp1 = cp.tile([H, BL], f32)
            hall = cp.tile([H, C * BL], f32)
            r = cp.tile([H, 2 * BL], f32)
            u = cp.tile([H, 2 * BL], f32)
            q = cp.tile([H, 2 * BL], f32)
            w = cp.tile([H, 2 * BL], f32)
            v = cp.tile([H, 2 * BL], f32)
            e1 = cp.tile([H, 2 * BL], f32)
            if zs:
                fe2 = cp.tile([H, 4 * BL], bf16)
                fp_, e2 = fe2[:, 0:BL], fe2[:, BL : 2 * BL]
            else:
                fp_ = cp.tile([H, BL], f32)
                e2 = cp.tile([H, BL], f32)
            hp2 = cp.tile([H, 2 * BL], f32)

            wfcT = wpack[:, WC_WFC : WC_WFC + O]
            bfc8 = wrows[0:1, 1024 : 1024 + 8 * O]

            if zero_hall:
                nc.gpsimd.memset(hall[:], 0.0)
            nc.sync.dma_start(wpack[:], wpack_d)
            nc.sync.dma_start(whhb[:], whhb_d)
            if not zs:
                # sim/debug path runs the recurrence in f32: upconvert whhb
                whhf = cp.tile([H, 3 * H], f32)
                nc.vector.tensor_scalar_add(whhf[:], whhb[:], 0.0)
                whhrT = whhf[:, 0:H]
                whhzT = whhf[:, H : 2 * H]
                whhnT = whhf[:, 2 * H : 3 * H]
            nc.sync.dma_start(wrows[:], wrows_d)
            nc.sync.dma_start(lcat[:], lcat_d)
            nc.vector.tensor_scalar_add(hp1[:], wpack[:, WC_HP1 : WC_HP1 + BL], 0.0)
            nc.vector.tensor_scalar_add(fp_[:], wpack[:, WC_HP1 : WC_HP1 + BL], 0.0)
            nc.vector.memzero(e2[:])
            if zs:
                nc.vector.tensor_scalar_add(
                    hp2[:, 0:BL], wpack[:, WC_HP1 : WC_HP1 + BL], 0.0)

            xa = cp.tile([2, SB], f32)
            xbf = cp.tile([1, SB], bf16)
            a_n0 = cp.tile([H, HS], f32)
            a_n1 = cp.tile([H, HS], f32)
            nc.gpsimd.memset(xa[:], 1.0)
            nc.sync.dma_start(xbf[:], xcols_d[0:1, :])
            nc.vector.tensor_scalar_add(xa[0:1, :], xbf[:], 0.0)

            with (
                tc.tile_pool(name="ps", bufs=1, space="PSUM") as psp,
            ):
                pst = {}
                for half, a_n in ((0, a_n0), (1, a_n1)):
                    pst[half] = tuple(
                        psp.tile([H, HS], f32, name=f"ps_{g}{half}")
                        for g in "rznt"
                    ) + (a_n,)

                def preload_mm(half, g):
                    ps = pst[half][g]
                    xh = xa[:, half * HS : (half + 1) * HS]
                    nc.tensor.matmul(
                        ps[:], lcat[:, g * H : (g + 1) * H], xh,
                        start=True, stop=True,
                    )

                def preload_copy(half, chunk, nchunk=2):
                    cw = HS // nchunk
                    csl = slice(chunk * cw, (chunk + 1) * cw)
                    nc.vector.tensor_scalar_add(
                        pst[half][4][:, csl], pst[half][3][:, csl], 0.0
                    )

                def preload_all(half):
                    for g in range(4):
                        preload_mm(half, g)
                    preload_copy(half, 0)
                    preload_copy(half, 1)

                # column 0's first half is preloaded before the loop
                preload_all(0)

                with tc.For_i(
                    0, n_cols if loop_cols is None else loop_cols, 1,
                    hint_engines=(FP.PE, FP.Activation, FP.DVE),
                ) as cv:

                    def steps(half, lo, hi, post=None):
                        ps_r, ps_z, ps_n, _, a_n = pst[half]
                        for t in range(lo, hi):
                            sl = slice(t * BL, (t + 1) * BL)
                            pb = slice((t % 2) * BL, (t % 2 + 1) * BL)
                            rr, uu, qq, ww, vv, ee1 = (
                                x[:, pb] for x in (r, u, q, w, v, e1))
                            pr, pw = t % 2, (t + 1) % 2
                            if zs:
                                fpw = fe2[:, pw * 2 * BL : pw * 2 * BL + BL]
                                e2w = fe2[:, pw * 2 * BL + BL : (pw + 1) * 2 * BL]
                                hpr = hp2[:, pr * BL : (pr + 1) * BL]
                                hpw = hp2[:, pw * BL : (pw + 1) * BL]
                                hp1v = fe2[
                                    :, pr * 2 * BL : (pr + 1) * 2 * BL
                                ].rearrange("p (a o) -> p a o", a=2)
                                outs = [
                                    bass.broadcast_tensor_aps(
                                        ps[:, sl].rearrange("p (a o) -> p a o", a=1),
                                        hp1v,
                                    )[0]
                                    for ps in (ps_r, ps_n, ps_z)
                                ]
                            else:
                                fpw, e2w, hpr, hpw = fp_[:], e2[:], hp1[:], hp1[:]
                                hp1v = hp1[:]
                                outs = [ps_r[:, sl], ps_n[:, sl], ps_z[:, sl]]
                            if zs:
                                wsel = (whhb[:, 0:H], whhb[:, 2 * H : 3 * H],
                                        whhb[:, H : 2 * H])
                            else:
                                wsel = (whhrT, whhnT, whhzT)
                            for o_, w_ in zip(outs, wsel):
                                nc.tensor.matmul(
                                    o_, w_, hp1v, start=False, stop=True,
                                    skip_group_check=True,
                                )
                            nc.scalar.activation(rr, ps_r[:, sl], AF.Sigmoid)
                            nc.scalar.activation(uu, ps_z[:, sl], AF.Sigmoid, scale=-1.0)
                            nc.vector.tensor_mul(qq, rr, ps_n[:, sl])
                            nc.vector.tensor_add(ww, qq, a_n[:, sl])
                            nc.scalar.activation(vv, ww, AF.Sigmoid, scale=2.0)
                            nc.vector.tensor_mul(ee1, uu, hpr)
                            nc.vector.tensor_sub(fpw, hpr, ee1)
                            nc.vector.scalar_tensor_tensor(
                                e2w, uu, 2.0, vv, op0=OPM, op1=OPM
                            )
                            nc.vector.tensor_add(hpw, fpw, e2w)
                            if post and t in post:
                                for thunk in post[t]:
                                    thunk()

                    # interleave half-1 preloads into half-0's step windows,
                    # and next column's x DMA + half-0 preloads into half-1's.
                    p0 = {
                        8 + i: [lambda g=i: preload_mm(1, g)] for i in range(4)
                    }
                    p0[13] = [lambda: preload_copy(1, 0)]
                    p0[15] = [lambda: preload_copy(1, 1)]
                    p1 = {
                        16: [lambda: nc.sync.dma_start(
                            xbf[:], xcols_d[bass.ds(cv + 1, 1), :]
                        )],
                        18: [lambda: nc.vector.tensor_scalar_add(
                            xa[0:1, :], xbf[:], 0.0
                        )],
                    }
                    for i in range(4):
                        p1[24 + 2 * i] = [lambda g=i: preload_mm(0, g)]
                    p1[34] = [lambda: preload_copy(0, 0)]
                    p1[36] = [lambda: preload_copy(0, 1)]

                    steps(0, 0, sph, post=p0)
                    steps(1, 0, sph, post=p1)
                    nc.vector.tensor_scalar_add(
                        hall[:, bass.ts(cv, BL)],
                        hp2[:, 0:BL] if zs else hp1[:], -1.0
                    )

            # output head: partial logits -> allreduce(fwd,bwd) -> softmax(relu(.))
            with (
                tc.tile_pool(name="fc", bufs=1) as fcp,
                tc.tile_pool(name="psfc", bufs=1, space="PSUM") as psfc,
                tc.tile_pool(name="dramp", bufs=1, space="DRAM") as dp,
            ):
                lps = psfc.tile([128, 8 * O], f32)
                for k in range(8):
                    nc.tensor.matmul(
                        lps[:, k * O : (k + 1) * O],
                        hall[:, k * 128 : (k + 1) * 128],
                        wfcT,
                        start=True,
                        stop=True,
                    )
                lsb = fcp.tile([128, 8 * O], f32)
                nc.vector.tensor_scalar_add(lsb[:], lps[:], 0.0)
                lloc = dp.tile([C * BL, O], f32)
                lred = dp.tile([C * BL, O], f32)
                nc.sync.dma_start(
                    lloc.rearrange("(k p) o -> p k o", p=128),
                    lsb[:].rearrange("p (k o) -> p k o", k=8),
                )
                if skip_collective:
                    nc.sync.dma_start(lred[:], lloc[:])
                else:
                    nc.gpsimd.collective_compute(
                        "AllReduce",
                        mybir.AluOpType.add,
                        replica_groups=[[0, 4], [1, 5], [2, 6], [3, 7]],
                        ins=[lloc.opt()],
                        outs=[lred.opt()],
                    )
                # bias broadcast: ones128^T (128,1) @ bfc8 (1, 512)
                bias_ps = psfc.tile([128, 8 * O], f32)
                ones128 = fcp.tile([1, H], f32)
                nc.gpsimd.memset(ones128[:], 1.0)
                nc.tensor.matmul(bias_ps[:], ones128[:], bfc8, start=True, stop=True)
                lsum = fcp.tile([128, 8 * O], f32)
                nc.sync.dma_start(
                    lsum[:].rearrange("p (k o) -> p k o", k=8),
                    lred.rearrange("(k p) o -> p k o", p=128),
                )
                lbi = fcp.tile([128, 8 * O], f32)
                nc.vector.tensor_add(lbi[:], lsum[:], bias_ps[:])
                ex = fcp.tile([128, 8 * O], f32)
                nc.scalar.activation(ex[:], lbi[:], AF.Exp)
                # exp(relu(x)) == max(1, exp(x))
                nc.vector.tensor_scalar_max(ex[:], ex[:], 1.0)
                sums = fcp.tile([128, 8], f32)
                nc.vector.tensor_reduce(
                    sums[:],
                    ex[:].rearrange("p (k o) -> p k o", k=8),
                    axis=mybir.AxisListType.X,
                    op=mybir.AluOpType.add,
                )
                rs = fcp.tile([128, 8], f32)
                nc.vector.reciprocal(rs[:], sums[:])
                osb = fcp.tile([128, 8 * O], bf16)
                for k in range(8):
                    nc.vector.tensor_scalar_mul(
                        osb[:, k * O : (k + 1) * O],
                        ex[:, k * O : (k + 1) * O],
                        rs[:, k : k + 1],
                    )
                nc.sync.dma_start(
                    out_d.rearrange("(k p) o -> p k o", p=128),
                    osb[:].rearrange("p (k o) -> p k o", k=8),
                )


_CACHE = {}


def _build():
    if "nc" not in _CACHE:
        nc = bacc.Bacc("TRN2", target_bir_lowering=False, debug=False, num_devices=NCORES)
        _emit(nc)
        nc.compile()
        _CACHE["nc"] = nc
    return _CACHE["nc"]


def _dir_prep(inputs, d):
    """Per-direction host prep shared by the 4 batch-group cores."""
    sfx = "f" if d == 0 else "b"
    Wih = inputs[f"Wih_{sfx}"][:, 0]
    Whh = inputs[f"Whh_{sfx}"]
    bih = inputs[f"bih_{sfx}"]
    bhh = inputs[f"bhh_{sfx}"]
    Wr, Wz, Wn = Whh[:H], Whh[H : 2 * H], Whh[2 * H :]
    lcat = np.zeros((2, 4 * H), np.float32)
    lcat[0, 0:H] = Wih[:H]
    lcat[1, 0:H] = bih[:H] + bhh[:H] - Wr.sum(1)
    lcat[0, H : 2 * H] = Wih[H : 2 * H]
    lcat[1, H : 2 * H] = bih[H : 2 * H] + bhh[H : 2 * H] - Wz.sum(1)
    lcat[1, 2 * H : 3 * H] = bhh[2 * H :] - Wn.sum(1)
    lcat[0, 3 * H : 4 * H] = Wih[2 * H :]
    lcat[1, 3 * H : 4 * H] = bih[2 * H :]
    wfc_half = inputs["W_fc"][:, :H] if d == 0 else inputs["W_fc"][:, H:]
    wpack = np.zeros((H, WP), np.float32)
    wpack[:, WC_WFC : WC_WFC + O] = wfc_half.T
    whhb = np.concatenate([Wr.T, Wz.T, Wn.T], axis=1).astype(ml_dtypes.bfloat16)
    wrows = np.zeros((1, 3 * 512), np.float32)
    wrows[0, 1024 : 1024 + 8 * O] = np.tile(inputs["b_fc"], 8)
    xT = np.transpose(inputs["x"], (2, 1, 0))  # (C, S, B) view
    if d == 1:
        xT = xT[:, ::-1, :]
    return {"wpack": wpack, "wrows": wrows, "lcat": lcat, "xT": xT,
            "whhb": whhb}


def _core_inputs(inputs, d, g, prep=None):
    """Host-side prep for core (direction d, batch group g)."""
    if prep is None:
        prep = _dir_prep(inputs, d)
    bsl = slice(g * BL, (g + 1) * BL)
    xcols_pad = np.zeros((C + 1, SB), ml_dtypes.bfloat16)
    xcols_pad[:C] = np.asarray(
        prep["xT"][:, :, bsl], dtype=ml_dtypes.bfloat16
    ).reshape(C, SB)
    wpack = prep["wpack"].copy()
    wpack[:, WC_HP1 : WC_HP1 + BL] = (inputs["h_prev"][d, bsl] + 1.0).T
    return {"xcols": xcols_pad, "wpack": wpack, "wrows": prep["wrows"],
            "lcat": prep["lcat"], "whhb": prep["whhb"]}


def kernel(**inputs) -> np.ndarray:
    inputs = {k: np.asarray(v, dtype=np.float32) for k, v in inputs.items()}
    nc = _build()
    preps = {d: _dir_prep(inputs, d) for d in (0, 1)}
    in_maps = []
    for core in range(NCORES):
        d, g = (0, core) if core < 4 else (1, core - 4)
        in_maps.append(_core_inputs(inputs, d, g, preps[d]))
    res = run_bass_kernel_spmd(nc, in_maps, core_ids=list(range(NCORES)))
    out = np.empty((B, C, O), np.float32)
    for g in range(4):
        o = res.results[g]["out"].astype(np.float32).reshape(C, BL, O)
        out[g * BL : (g + 1) * BL] = np.transpose(o, (1, 0, 2))
    return out

